# revision 57
# baseline (speedup 1.0000x reference)
"""Trainium2 Bass kernel for nn_MultiHeadCrossAttention (ragged kv cross-attention).

Contract: kernel(**inputs) takes FULL numpy inputs, shards across 8 NeuronCores
(data-parallel: core i handles batch i//2, query rows [(i%2)*2048, +2048)),
runs one SPMD Bass/Tile kernel, gathers the FULL output.

Per-core pipeline (all matmuls bf16, fp32 accumulation):
  P1: kT = rms-normed (kv_w_k.T @ condT) in head-padded [d,h,m] layout;
      v [m,(h,73)] with a denominator-ones column; ragged kv handled by
      zeroing condT columns and the ones column beyond kv_seqlen (zero k gives
      exp(0)=1 whose V-row contribution and den contribution are both 0).
  P2: qT = rms-normed (x @ q_w).T, head-padded [d,h,q].  Query blocks 0-1 are
      projected standalone; blocks 2-3's tiles are interleaved into P3's first
      two attention blocks (sharing the out-proj PSUM ring) so attention
      matmuls fill the rmsnorm chain's stall gaps.
  P3: scoresT[m,q] = kT_h.T @ qT_h; exp on ScalarE (no max-subtraction; scores
      are bounded by the rms norms); PV: out[q,(h,73)] += expT.T @ v_aug;
      normalize by the den column; DMA-transpose to attnT.
  P4 (interleaved with P3 per query block): out = attnT.T @ proj_w.

Head-padded transposes: the rmsnorm multiply writes normalized rows into a
[128, H*128] staging tile with head h at columns 128h..128h+72 (the other 56
columns are junk).  One DMA transpose then lands head h's dims at partitions
0..72 of free-slot h — the layout score/PV matmuls need — with no per-head
padding DMAs.

Dependency-chain discipline: PSUM's only reader is an immediate ScalarE copy
(psum banks never wait on the norm chain); each tile's ln/exp/mults/transpose
are emitted one tile late so every cross-engine wait has a tile of slack; all
activation-table loads are pinned to the one act-func set that contains
square+ln+exp (single table load, no thrash).
"""

import numpy as np
import ml_dtypes

import concourse.bacc as bacc
import concourse.bass as bass
import concourse.mybir as mybir
import concourse.tile as tile
from concourse.bass_utils import run_bass_kernel_spmd

BF16 = ml_dtypes.bfloat16
AF = mybir.ActivationFunctionType
ALU = mybir.AluOpType

B, N, M, C = 4, 4096, 512, 1152
H, D = 16, 72
NCORES = 8
NL = N * B // NCORES          # 2048 query rows per core
KC = C // 128                 # 9 contraction chunks
QT = NL // 128                # 16 query tiles per core
NB = NL // 512                # 4 query blocks of 512
MC = M // 128                 # 4 kv chunks
EPS = 1e-6
HB = 4                        # heads per PV group (one PSUM bank: 4*73 <= 512)
NHG = H // HB

# head-aligned output blocks of C: 7 + 7 + 2 heads
HBLOCKS = [(0, 504, 7), (504, 504, 7), (1008, 144, 2)]

_BUILD_CACHE = {}


def _pin_act_tables(nc):
    """Rewrite every InstLoadActFuncSet to the single act-func set that covers
    all activation functions used, and delete the now-redundant loads (the CFG
    is a linear chain, so the first load dominates all activations)."""
    import concourse.hw_specs as hw_specs
    try:
        tables = hw_specs.get_activation_tables(nc.m.arch)
    except Exception:
        return
    name = "natural_log_exp_and_others"
    if name not in tables:
        return
    idx = list(tables).index(name)
    allowed = tables[name]
    used = set()
    for b in nc.main_func.blocks:
        for i in b.instructions:
            if isinstance(i, mybir.InstActivation):
                used.add(i.func)
    if not used <= allowed:
        return
    first = True
    for b in nc.main_func.blocks:
        il = b.instructions
        for i in [x for x in il if isinstance(x, mybir.InstLoadActFuncSet)]:
            if first:
                i.act_func_set_id = idx
                first = False
            else:
                il.remove(i)


def _build(with_bias: bool, uniform_scale, key):
    if key in _BUILD_CACHE:
        return _BUILD_CACHE[key]

    nc = bacc.Bacc("TRN2", target_bir_lowering=False, debug=False,
                   num_devices=NCORES)
    f32, bf16 = mybir.dt.float32, mybir.dt.bfloat16

    xTt = nc.dram_tensor("xTt", [QT, 128, KC, 128], bf16, kind="ExternalInput").ap()
    condT = nc.dram_tensor("condT", [MC, 128, KC, 128], bf16, kind="ExternalInput").ap()
    qw = nc.dram_tensor("qw", [128, KC, C], bf16, kind="ExternalInput").ap()
    kvw = nc.dram_tensor("kvw", [128, KC, 2 * C], bf16, kind="ExternalInput").ap()
    pw = nc.dram_tensor("pw", [128, KC, C], bf16, kind="ExternalInput").ap()
    vones = nc.dram_tensor("vones", [M, H], bf16, kind="ExternalInput").ap()
    if uniform_scale is None:
        wkp = nc.dram_tensor("wkp", [128, H], f32, kind="ExternalInput").ap()
    if with_bias:
        qb = nc.dram_tensor("qb", [1, C], bf16, kind="ExternalInput").ap()
        kvb = nc.dram_tensor("kvb", [1, 2 * C], bf16, kind="ExternalInput").ap()
        pb = nc.dram_tensor("pb", [1, C], bf16, kind="ExternalInput").ap()
        maskv = nc.dram_tensor("maskv", [1, M], bf16, kind="ExternalInput").ap()
    out = nc.dram_tensor("out", [NL, C], f32, kind="ExternalOutput").ap()

    exp_scale = float(uniform_scale) if uniform_scale is not None else 1.0

    def bcast(ap_scalar, groups, width):
        # [128, groups] -> [128, groups, width] with stride-0 inner dim
        return bass.AP(tensor=ap_scalar.tensor, offset=ap_scalar.offset,
                       ap=[ap_scalar.ap[0], [ap_scalar.ap[1][0], groups], [0, width]])

    with tile.TileContext(nc) as tc:
        from contextlib import ExitStack
        es_top = ExitStack()   # left: persist + smallp (whole run)
        es_p1 = ExitStack()    # left: P1 staging + psum
        es_qtp = ExitStack()   # left: qTp ring (P2..end)
        es_p2q = ExitStack()   # left: q staging (P2..mid-P3)
        es_p2ps = ExitStack()  # psum: standalone-P2 ring
        es_pw = ExitStack()    # left: proj weights (P2..end)
        es_p3 = ExitStack()    # left + psum: P3 pools
        es_w = ExitStack()     # right: q/x inputs (start..mid-P3)
        es_kvw = ExitStack()   # right: kv weights (start..P1 end)

        persist = es_top.enter_context(tc.tile_pool(name="persist", bufs=1))
        smallp = es_top.enter_context(tc.tile_pool(name="smallp", bufs=4))

        kTp = persist.tile([128, H, M], bf16)          # padded kT [d<=72, h, m]
        v16 = persist.tile([128, MC, H * 73], bf16)    # v + den-ones col per head
        eps_sb = persist.tile([128, 1], f32)
        nc.vector.memset(eps_sb, EPS)
        if with_bias:
            ones1 = persist.tile([1, 128], bf16)
            nc.vector.memset(ones1, 1.0)
            qb_sb = persist.tile([1, C], bf16)
            kvb_sb = persist.tile([1, 2 * C], bf16)
            pb_sb = persist.tile([1, C], bf16)
            maskv_sb = persist.tile([1, M], bf16)
            nc.sync.dma_start(out=qb_sb, in_=qb)
            nc.sync.dma_start(out=kvb_sb, in_=kvb)
            nc.sync.dma_start(out=pb_sb, in_=pb)
            nc.sync.dma_start(out=maskv_sb, in_=maskv)
        if uniform_scale is None:
            wkp_sb = persist.tile([128, H], f32)
            nc.sync.dma_start(out=wkp_sb, in_=wkp)

        # ===== weight / input pools (right stack: close early) =====
        wpool = es_w.enter_context(
            tc.tile_pool(name="wpool", bufs=1, side="right"))
        p2x = es_w.enter_context(
            tc.tile_pool(name="p2x", bufs=3, side="right"))
        qw_sb = wpool.tile([128, KC, C], bf16)
        kvwpool = es_kvw.enter_context(
            tc.tile_pool(name="kvwpool", bufs=1, side="right"))
        kvw_sb = kvwpool.tile([128, KC, 2 * C], bf16)

        # ====== Phase 2 machinery (tiles interleaved into P1 and P3) ======
        qTnbpool = es_qtp.enter_context(tc.tile_pool(name="qTnb", bufs=3))
        p2q = es_p2q.enter_context(tc.tile_pool(name="p2q", bufs=2))
        qTp_nbs = [None] * NB
        qtail = [None]
        prev_proj = [None]   # chain shared-ring psum accumulation groups

        def q_rms_tail(pending):
            if pending is None:
                return
            qt, qTp_nb, q16p, qf32s, ssqq, tdep = pending
            j = qt % 4
            lnq = smallp.tile([128, H], f32, tag="lnq", name=f"lnq{qt}")
            nc.scalar.activation(lnq, ssqq, AF.Ln, bias=eps_sb, scale=1.0 / D)
            invq = smallp.tile([128, H], f32, tag="invq", name=f"invq{qt}")
            nc.scalar.activation(invq, lnq, AF.Exp, scale=-0.5)
            for bi, (bo, bw, hh) in enumerate(HBLOCKS):
                h0 = bo // D
                nc.vector.tensor_tensor(
                    out=q16p[:, h0:h0 + hh, 0:D],
                    in0=qf32s[bi][:, 0:bw].rearrange("p (h d) -> p h d", h=hh),
                    in1=bcast(invq[:, h0:h0 + hh], hh, D), op=ALU.mult)
            tr = nc.sync.dma_start(out=qTp_nb[:, :, j * 128:(j + 1) * 128],
                                   in_=q16p.rearrange("p h c -> p (h c)"),
                                   transpose=True)
            if tdep is not None:
                # ring-slot reuse: qTp3 aliases qTp0's buffer; the transpose
                # must not start until block-0 scores finished reading it
                tile.add_dep_helper(tr.ins, tdep.ins, sync=True,
                                    reason="qTp ring reuse WAR")

        def qproj_tile(qt, qTp_nb, psum_pool, psum_tag, chain,
                       sq_on_dve=False, tdep=None):
            j = qt % 4
            xt = p2x.tile([128, KC, 128], bf16, tag="xt", name=f"xt{qt}")
            (nc.gpsimd if qt % 2 == 0 else nc.scalar).dma_start(
                out=xt, in_=xTt[qt])
            q16p = p2q.tile([128, H, 128], bf16, tag="q16",
                            name=f"q16_{qt}", bufs=3)
            if qt < 3:
                nc.gpsimd.memset(q16p, 0.0)
            qf32s = []
            ssqq = smallp.tile([128, H], f32, tag="ssqq", name=f"ssqq{qt}")
            for bi, (bo, bw, hh) in enumerate(HBLOCKS):
                h0 = bo // D
                psQ = psum_pool.tile([128, 512], f32, tag=psum_tag,
                                     name=f"psQ{qt}_{bi}", bufs=2)
                mm_first = mm_last = None
                for kc in range(KC):
                    mm = nc.tensor.matmul(
                        psQ[:, 0:bw], xt[:, kc, :], qw_sb[:, kc, bo:bo + bw],
                        start=(kc == 0),
                        stop=(kc == KC - 1 and not with_bias))
                    if mm_first is None:
                        mm_first = mm
                    mm_last = mm
                if with_bias:
                    mm_last = nc.tensor.matmul(psQ[:, 0:bw], ones1,
                                               qb_sb[:, bo:bo + bw],
                                               start=False, stop=True)
                if chain is not None:
                    if chain[0] is not None:
                        tile.add_dep_helper(mm_first.ins, chain[0].ins,
                                            sync=False,
                                            reason="shared psum ring order")
                    chain[0] = mm_last
                qf32 = p2q.tile([128, 504], f32, tag="qf32",
                                name=f"qf32_{qt}_{bi}", bufs=3)
                nc.scalar.activation(qf32[:, 0:bw], psQ[:, 0:bw], AF.Copy)
                qf32s.append(qf32)
                qsq = smallp.tile([128, 504], bf16, tag="qsq",
                                  name=f"qsq{qt}_{bi}")
                if sq_on_dve:
                    nc.vector.tensor_tensor(out=qsq[:, 0:bw],
                                            in0=qf32[:, 0:bw],
                                            in1=qf32[:, 0:bw], op=ALU.mult)
                else:
                    nc.scalar.activation(qsq[:, 0:bw], qf32[:, 0:bw],
                                         AF.Square)
                if bi == 0:
                    # previous tile's ln/exp/mults/transpose: emitted here so
                    # every cross-engine wait has ~a tile of slack
                    q_rms_tail(qtail[0])
                    qtail[0] = None
                nc.vector.tensor_reduce(
                    ssqq[:, h0:h0 + hh],
                    qsq[:, 0:bw].rearrange("p (h d) -> p h d", h=hh),
                    axis=mybir.AxisListType.X, op=ALU.add)
            qtail[0] = (qt, qTp_nb, q16p, qf32s, ssqq, tdep)


        # ================= Phase 1: K/V projection =================
        p1k = es_p1.enter_context(tc.tile_pool(name="p1k", bufs=2))
        p1ct = es_p1.enter_context(tc.tile_pool(name="p1ct", bufs=4))
        p1ps = es_p1.enter_context(
            tc.tile_pool(name="p1ps", bufs=1, space="PSUM"))

        # prefetch order: condT + K weights first (first matmuls), then V, Q
        condTs = []
        for mc in range(MC):
            ct = p1ct.tile([128, KC, 128], bf16, tag="ct", name=f"ct{mc}")
            nc.gpsimd.dma_start(out=ct, in_=condT[mc])
            condTs.append(ct)
        for bo, bw, _ in HBLOCKS:
            nc.sync.dma_start(out=kvw_sb[:, :, bo:bo + bw],
                              in_=kvw[:, :, bo:bo + bw])
        for vo, vw, _ in HBLOCKS:
            nc.gpsimd.dma_start(out=kvw_sb[:, :, C + vo:C + vo + vw],
                                in_=kvw[:, :, C + vo:C + vo + vw])
        for bo, bw, _ in HBLOCKS:
            nc.gpsimd.dma_start(out=qw_sb[:, :, bo:bo + bw],
                                in_=qw[:, :, bo:bo + bw])

        qTp_nbs[0] = qTnbpool.tile([128, H, 512], bf16, tag="qTp",
                                   name="qTp0")
        ktail = [None]

        def k_rms_tail(pending):
            if pending is None:
                return
            mc, k16p, kf32s, ssqk = pending
            msl = slice(mc * 128, (mc + 1) * 128)
            lnk = smallp.tile([128, H], f32, tag="lnk", name=f"lnk{mc}")
            nc.scalar.activation(lnk, ssqk, AF.Ln, bias=eps_sb, scale=1.0 / D)
            invk = smallp.tile([128, H], f32, tag="invk", name=f"invk{mc}")
            nc.scalar.activation(invk, lnk, AF.Exp, scale=-0.5)
            for bi, (bo, bw, hh) in enumerate(HBLOCKS):
                h0 = bo // D
                nc.vector.tensor_tensor(
                    out=k16p[:, h0:h0 + hh, 0:D],
                    in0=kf32s[bi][:, 0:bw].rearrange("p (h d) -> p h d", h=hh),
                    in1=bcast(invk[:, h0:h0 + hh], hh, D), op=ALU.mult)
            nc.sync.dma_start(out=kTp[:, :, msl],
                              in_=k16p.rearrange("p h c -> p (h c)"),
                              transpose=True)

        for mc in range(MC):
            msl = slice(mc * 128, (mc + 1) * 128)
            condT_sb = condTs[mc]
            # head-padded staging: head h at cols 128h..128h+72 (rest junk)
            k16p = p1k.tile([128, H, 128], bf16, tag="k16", name=f"k16_{mc}",
                            bufs=3)
            if mc < 3:
                nc.gpsimd.memset(k16p, 0.0)
            kf32s = []
            ssqk = smallp.tile([128, H], f32, tag="ssqk", name=f"ssqk{mc}")
            for bi, (bo, bw, hh) in enumerate(HBLOCKS):
                h0 = bo // D
                psK = p1ps.tile([128, 512], f32, tag="ppk",
                                name=f"psK{mc}_{bi}", bufs=5)
                for kc in range(KC):
                    nc.tensor.matmul(
                        psK[:, 0:bw], condT_sb[:, kc, :],
                        kvw_sb[:, kc, bo:bo + bw],
                        start=(kc == 0),
                        stop=(kc == KC - 1 and not with_bias))
                if with_bias:
                    nc.tensor.matmul(psK[:, 0:bw], maskv_sb[:, msl],
                                     kvb_sb[:, bo:bo + bw],
                                     start=False, stop=True)
                # the only psum reader: frees the bank without waiting on
                # the norm chain
                kf32 = p1k.tile([128, 504], f32, tag="kf32",
                                name=f"kf32_{mc}_{bi}", bufs=4)
                nc.scalar.activation(kf32[:, 0:bw], psK[:, 0:bw], AF.Copy)
                kf32s.append(kf32)
                ksq = smallp.tile([128, 504], bf16, tag="ksq",
                                  name=f"ksq{mc}_{bi}")
                nc.scalar.activation(ksq[:, 0:bw], kf32[:, 0:bw], AF.Square)
                if bi == 0:
                    k_rms_tail(ktail[0])
                    ktail[0] = None
                nc.vector.tensor_reduce(
                    ssqk[:, h0:h0 + hh],
                    ksq[:, 0:bw].rearrange("p (h d) -> p h d", h=hh),
                    axis=mybir.AxisListType.X, op=ALU.add)
            ktail[0] = (mc, k16p, kf32s, ssqk)

            for vi, (vo, vw, hh) in enumerate(HBLOCKS):
                psV = p1ps.tile([128, 512], f32, tag="ppv",
                                name=f"psV{mc}_{vi}", bufs=3)
                for kc in range(KC):
                    nc.tensor.matmul(
                        psV[:, 0:vw], condT_sb[:, kc, :],
                        kvw_sb[:, kc, C + vo:C + vo + vw],
                        start=(kc == 0),
                        stop=(kc == KC - 1 and not with_bias))
                if with_bias:
                    nc.tensor.matmul(psV[:, 0:vw], maskv_sb[:, msl],
                                     kvb_sb[:, C + vo:C + vo + vw],
                                     start=False, stop=True)
                h0 = vo // D
                vdst = v16[:, mc, 73 * h0:73 * (h0 + hh)]
                nc.vector.tensor_copy(
                    out=vdst.rearrange("p (h t) -> p h t", h=hh)[:, :, 0:D],
                    in_=psV[:, 0:vw].rearrange("p (h d) -> p h d", h=hh))
            nc.gpsimd.dma_start(
                out=v16[:, mc, :].rearrange("p (h t) -> p h t",
                                            h=H)[:, :, 72:73],
                in_=vones[msl, :])
        k_rms_tail(ktail[0])
        ktail[0] = None

        if uniform_scale is None:
            # rare path: per-(head,dim) qn*kn scale applied to padded kT rows
            for h in range(H):
                nc.vector.tensor_scalar_mul(
                    kTp[0:D, h, :], kTp[0:D, h, :], wkp_sb[0:D, h:h + 1])
        es_p1.close()
        es_kvw.close()

        # ---- standalone P2: query block 0 ----
        p2ps = es_p2ps.enter_context(
            tc.tile_pool(name="p2ps", bufs=7, space="PSUM"))
        for j in range(4):
            qproj_tile(j, qTp_nbs[0], p2ps, "pq", None)
        q_rms_tail(qtail[0])
        qtail[0] = None
        es_p2ps.close()

        # proj weights: loaded while P3-nb0 runs (first use is nb1)
        pwpool = es_pw.enter_context(tc.tile_pool(name="pwpool", bufs=1))
        pw_sb = pwpool.tile([128, KC, C], bf16)
        nc.gpsimd.dma_start(out=pw_sb[:, :, 0:504], in_=pw[:, :, 0:504])
        nc.scalar.dma_start(out=pw_sb[:, :, 504:1008], in_=pw[:, :, 504:1008])
        nc.gpsimd.dma_start(out=pw_sb[:, :, 1008:1152], in_=pw[:, :, 1008:1152])

        # ===== Phases 3+4: attention + q-proj blocks 2-3 + out projection ===
        p3exp = es_p3.enter_context(tc.tile_pool(name="p3exp", bufs=HB + 1))
        p3attn = es_p3.enter_context(tc.tile_pool(name="p3attn", bufs=5))
        attnTnb = es_p3.enter_context(tc.tile_pool(name="attnTnb", bufs=2))
        p4o = es_p3.enter_context(tc.tile_pool(name="p4o", bufs=3))
        p3psS = es_p3.enter_context(
            tc.tile_pool(name="p3psS", bufs=2, space="PSUM"))
        p3psPV = es_p3.enter_context(
            tc.tile_pool(name="p3psPV", bufs=2, space="PSUM"))
        p4ps = es_p3.enter_context(
            tc.tile_pool(name="p4ps", bufs=2, space="PSUM"))

        attnTs = [None] * NB
        last_score_mm = [None] * NB

        def proj_qt(pnb, j):
            qt = pnb * 4 + j
            for bo, bw, _ in HBLOCKS:
                psP = p4ps.tile([128, 512], f32, tag="psP",
                                name=f"psP_{qt}_{bo}")
                first = last = None
                for kc in range(KC):
                    mm = nc.tensor.matmul(
                        psP[:, 0:bw],
                        attnTs[pnb][:, kc, j * 128:(j + 1) * 128],
                        pw_sb[:, kc, bo:bo + bw],
                        start=(kc == 0),
                        stop=(kc == KC - 1 and not with_bias))
                    if first is None:
                        first = mm
                    last = mm
                if with_bias:
                    last = nc.tensor.matmul(psP[:, 0:bw], ones1,
                                            pb_sb[:, bo:bo + bw],
                                            start=False, stop=True)
                if prev_proj[0] is not None:
                    tile.add_dep_helper(first.ins, prev_proj[0].ins,
                                        sync=False,
                                        reason="shared psum ring order")
                prev_proj[0] = last
                so = p4o.tile([128, 512], f32, tag="so", name=f"so_{qt}_{bo}")
                nc.vector.tensor_copy(so[:, 0:bw], psP[:, 0:bw])
                nc.gpsimd.dma_start(
                    out=out[qt * 128:(qt + 1) * 128, bo:bo + bw],
                    in_=so[:, 0:bw])

        for nb in range(NB):
            if nb == 0:
                qTp_nbs[1] = qTnbpool.tile([128, H, 512], bf16, tag="qTp",
                                           name="qTp1")
                qTp_nbs[2] = qTnbpool.tile([128, H, 512], bf16, tag="qTp",
                                           name="qTp2")
            elif nb == 1:
                qTp_nbs[3] = qTnbpool.tile([128, H, 512], bf16, tag="qTp",
                                           name="qTp3")
            qTp_nb = qTp_nbs[nb]
            attnT = attnTnb.tile([128, KC, 512], bf16, tag="attnT",
                                 name=f"attnT{nb}")
            attnTs[nb] = attnT
            attnN = [p3attn.tile([128, C], bf16, tag="attnN",
                                 name=f"attnN_{nb}_{j}")
                     for j in range(4)]
            for hg in range(NHG):
                if nb == 0:
                    # two interleaved q-projection tiles (blocks 1-2)
                    for qt in (4 + 2 * hg, 5 + 2 * hg):
                        qproj_tile(qt, qTp_nbs[qt // 4], p4ps, "psP",
                                   prev_proj, sq_on_dve=True)
                elif nb == 1:
                    qproj_tile(12 + hg, qTp_nbs[3], p4ps, "psP",
                               prev_proj, sq_on_dve=True,
                               tdep=last_score_mm[0])
                if nb > 0:
                    proj_qt(nb - 1, hg)
                expts = []
                for hl in range(HB):
                    h = hg * HB + hl
                    expt = p3exp.tile([128, 2048], bf16, tag="expt",
                                      name=f"expt_{nb}_{h}")
                    for half in range(2):
                        psS = p3psS.tile([128, 1024], f32, tag="psS",
                                         name=f"psS_{nb}_{h}_{half}")
                        for mcl in range(2):
                            mc = half * 2 + mcl
                            smm = nc.tensor.matmul(
                                psS[:, mcl * 512:(mcl + 1) * 512],
                                kTp[0:D, h, mc * 128:(mc + 1) * 128],
                                qTp_nb[0:D, h, :],
                                start=True, stop=True)
                            last_score_mm[nb] = smm
                        nc.scalar.activation(
                            expt[:, half * 1024:(half + 1) * 1024],
                            psS, AF.Exp, scale=exp_scale)
                    expts.append(expt)
                for j in range(4):
                    # HB heads share one PSUM bank; accumulation groups
                    # are chained in emission order (start=True clears
                    # the whole bank's has_written bits).
                    psPV = p3psPV.tile([128, 512], f32, tag="psPV",
                                       name=f"psPV_{nb}_{hg}_{j}")
                    prev_last = None
                    for hl in range(HB):
                        h = hg * HB + hl
                        first = last = None
                        for mc in range(MC):
                            mm = nc.tensor.matmul(
                                psPV[:, hl * 73:(hl + 1) * 73],
                                expts[hl][:, mc * 512 + j * 128:
                                          mc * 512 + (j + 1) * 128],
                                v16[:, mc, 73 * h:73 * h + 73],
                                start=(mc == 0), stop=(mc == MC - 1))
                            if first is None:
                                first = mm
                            last = mm
                        if prev_last is not None:
                            tile.add_dep_helper(
                                first.ins, prev_last.ins, sync=False,
                                reason="psum-bank accum group order")
                        prev_last = last
                    pv3 = bass.AP(
                        tensor=psPV.tensor, offset=psPV.offset,
                        ap=[psPV.ap[0], [73, HB], [1, 73]])
                    rec = smallp.tile([128, HB], f32, tag="rec",
                                      name=f"rec_{nb}_{hg}_{j}")
                    nc.vector.reciprocal(rec, pv3[:, :, 72])
                    nc.vector.tensor_tensor(
                        out=attnN[j][:, hg * HB * D:(hg + 1) * HB * D]
                            .rearrange("p (h d) -> p h d", h=HB),
                        in0=pv3[:, :, 0:D],
                        in1=bcast(rec, HB, D), op=ALU.mult)
                    if hg == NHG - 1:
                        tr = nc.sync.dma_start(
                            out=attnT[:, :, j * 128:(j + 1) * 128],
                            in_=attnN[j], transpose=True)
                        if nb >= 2 and prev_proj[0] is not None:
                            # attnT ring reuse: don't overwrite attnT(nb-2)
                            # until its projection matmuls finished reading
                            tile.add_dep_helper(tr.ins, prev_proj[0].ins,
                                                sync=True,
                                                reason="attnT ring reuse WAR")
            if nb == 1:
                # last interleaved q tile done: flush its norm tail, release
                # the q weights and x staging
                q_rms_tail(qtail[0])
                qtail[0] = None
                es_w.close()
            if nb == NB - 1:
                for j in range(4):
                    proj_qt(nb, j)

        es_p3.close()
        es_pw.close()
        es_p2q.close()
        es_qtp.close()
        es_top.close()

    orig_pass = nc.insert_act_table_loads

    def _patched_act_pass():
        orig_pass()
        _pin_act_tables(nc)

    nc.insert_act_table_loads = _patched_act_pass
    nc.compile()
    _BUILD_CACHE[key] = nc
    return nc


def kernel(x, cond, kv_seqlen, q_w, q_b, kv_w, kv_b, proj_w, proj_b, qn_w, kn_w):
    x = np.asarray(x); cond = np.asarray(cond)
    kv_seqlen = np.asarray(kv_seqlen)
    q_w = np.asarray(q_w, np.float32); q_b = np.asarray(q_b, np.float32)
    kv_w = np.asarray(kv_w, np.float32); kv_b = np.asarray(kv_b, np.float32)
    proj_w = np.asarray(proj_w, np.float32); proj_b = np.asarray(proj_b, np.float32)
    qn_w = np.asarray(qn_w, np.float32); kn_w = np.asarray(kn_w, np.float32)

    with_bias = bool(np.any(q_b) or np.any(kv_b) or np.any(proj_b))
    qk = (qn_w * kn_w).astype(np.float64)
    if np.all(qk == qk[0]):
        uniform_scale = float(qk[0]) / float(np.sqrt(D))
    else:
        uniform_scale = None
    key = (with_bias, uniform_scale)
    nc = _build(with_bias, uniform_scale, key)

    def blocked_w(w):  # [C, dout] -> [128, KC, dout]
        return np.ascontiguousarray(
            w.reshape(KC, 128, -1).transpose(1, 0, 2)).astype(BF16)

    qwb = blocked_w(q_w)
    kvwb = blocked_w(kv_w)
    pwb = blocked_w(proj_w)
    if uniform_scale is None:
        # per-(d, h) scale for padded kT rows: qn*kn/sqrt(D)
        wk2 = (qn_w * kn_w).astype(np.float32) / np.float32(np.sqrt(D))
        wkpb = np.zeros((128, H), np.float32)
        wkpb[:D, :] = wk2[:, None]

    in_maps = []
    for core in range(NCORES):
        b, ns = core // 2, (core % 2) * NL
        A = x[b, ns:ns + NL, :].astype(np.float32)
        xtt = np.ascontiguousarray(
            A.reshape(QT, 128, KC, 128).transpose(0, 3, 2, 1)).astype(BF16)
        sl = int(kv_seqlen[b])
        ct = cond[b].astype(np.float32).T.copy()       # [C, M]
        ct[:, sl:] = 0.0
        ctb = np.ascontiguousarray(
            ct.reshape(KC, 128, MC, 128).transpose(2, 1, 0, 3)).astype(BF16)
        valid = (np.arange(M) < sl)
        vob = np.ascontiguousarray(
            np.repeat(valid[:, None], H, axis=1)).astype(BF16)
        m = {"xTt": xtt, "condT": ctb, "qw": qwb, "kvw": kvwb, "pw": pwb,
             "vones": vob}
        if uniform_scale is None:
            m["wkp"] = wkpb
        if with_bias:
            m["qb"] = q_b[None, :].astype(BF16)
            m["kvb"] = kv_b[None, :].astype(BF16)
            m["pb"] = proj_b[None, :].astype(BF16)
            m["maskv"] = valid[None, :].astype(BF16)
        in_maps.append(m)

    res = run_bass_kernel_spmd(nc, in_maps, core_ids=list(range(NCORES)))
    kernel._last_results = res

    out = np.empty((B, N, C), np.float32)
    for core in range(NCORES):
        b, ns = core // 2, (core % 2) * NL
        out[b, ns:ns + NL, :] = res.results[core]["out"]
    return out
